# revision 4
# baseline (speedup 1.0000x reference)
"""Trainium2 Bass kernel for multi-head quadratic spatial attention.

Problem: q,k,v [b=8, heads=8, h=32, w=32, d=64] fp32; full attention over
the 1024-position spatial grid independently per (b, head); output
[b, h, w, heads*d].

Sharding: data-parallel over batch — core c handles b=c (8 heads of
[1024, 64] attention per core), no cross-core communication.

Per-core pipeline (all matmuls in float32r — full PE rate, fp32 data):
  - load Q,K,V natural [1024, 64] as [128, 8, 64] SBUF tiles
  - PE-transpose Q,K tiles to d-major Qt,Kt [64, 1024] (identity matmul)
  - mm1: lhsT = Kt j-block [64, 128], rhs = Qt [64, 512] -> St PSUM
    [128 j, 1024 i]  (St = S^T so softmax reduction lands on PE, not DVE)
  - ScalarE exp(St * d^-1/2) -> Pt SBUF (scale folded into ACTIVATE)
  - mm2: lhsT = [V | 1] j-chunk [128, 65], rhs = Pt -> accumulate PSUM
    Ot [65, 1024]; row 64 = softmax denominators
  - PE-transpose Ot back per i-block, DVE reciprocal + tensor_scalar mult,
    DMA out natural layout
"""

from contextlib import ExitStack

import numpy as np

F32 = None  # set lazily in _imports
F32R = None

_cache = {}


def _imports():
    global F32, F32R
    import concourse.bass as bass
    import concourse.tile as tile
    from concourse import mybir
    from concourse.masks import make_identity

    F32 = mybir.dt.float32
    F32R = mybir.dt.float32r
    return bass, tile, mybir, make_identity


def _split_multi_waits(nc, mybir):
    """Walrus in this container supports only ONE sync-wait per instruction.
    Hoist extra waits onto same-engine InstNoOp's inserted just before."""
    ctr = 0
    for f in nc.m.functions:
        for bb in f.blocks:
            insts = bb.instructions
            if not any(
                i.sync_info and i.sync_info.on_wait and len(i.sync_info.on_wait) > 1
                for i in insts
            ):
                continue
            out = []
            for inst in insts:
                si = inst.sync_info
                waits = list(si.on_wait) if si and si.on_wait else []
                if len(waits) > 1:
                    for w in waits[:-1]:
                        ctr += 1
                        nop = mybir.InstNoOp(
                            name=f"I-wsplit-{ctr}",
                            engine=inst.engine,
                            ins=[],
                            outs=[],
                            sync_info=mybir.SyncInfo(on_wait=[w], on_update=[]),
                        )
                        nc.register_instruction(nop)
                        out.append(nop)
                    si.on_wait = waits[-1:]
                out.append(inst)
            bb.instructions = out


def _build_nc(heads=8, seq=1024, d=64):
    bass, tile, mybir, make_identity = _imports()
    nt = seq // 128
    dv = d + 1
    scale = float(d) ** -0.5
    n512 = max(1, seq // 512)
    nw = min(512, seq)

    nc = bass.Bass(trn_type="TRN2", target_bir_lowering=False)
    q_d = nc.dram_tensor("q", [heads, seq, d], F32, kind="ExternalInput")
    k_d = nc.dram_tensor("k", [heads, seq, d], F32, kind="ExternalInput")
    v_d = nc.dram_tensor("v", [heads, seq, d], F32, kind="ExternalInput")
    o_d = nc.dram_tensor("out", [seq, heads * d], F32, kind="ExternalOutput")

    q_ap = q_d[:].rearrange("n (t p) d -> n p t d", p=128)
    k_ap = k_d[:].rearrange("n (t p) d -> n p t d", p=128)
    v_ap = v_d[:].rearrange("n (t p) d -> n p t d", p=128)
    o_ap = o_d[:].rearrange("(t p) c -> p t c", p=128)

    with tile.TileContext(nc) as tc, ExitStack() as ctx:
        consts = ctx.enter_context(tc.tile_pool(name="consts", bufs=1))
        nat = ctx.enter_context(tc.tile_pool(name="nat", bufs=2))
        dmaj = ctx.enter_context(tc.tile_pool(name="dmaj", bufs=2))
        ptp = ctx.enter_context(tc.tile_pool(name="ptp", bufs=3))
        otp = ctx.enter_context(tc.tile_pool(name="otp", bufs=2))
        outp = ctx.enter_context(tc.tile_pool(name="outp", bufs=2))
        small = ctx.enter_context(tc.tile_pool(name="small", bufs=4))

        tp_ps = ctx.enter_context(tc.tile_pool(name="tp_ps", bufs=2, space="PSUM"))
        st_ps = ctx.enter_context(tc.tile_pool(name="st_ps", bufs=2, space="PSUM"))
        oa_ps = ctx.enter_context(tc.tile_pool(name="oa_ps", bufs=1, space="PSUM"))

        ident = consts.tile([128, 128], F32)
        make_identity(nc, ident[:])

        for n in range(heads):
            q_nat = nat.tile([128, nt, d], F32, tag="q_nat")
            k_nat = nat.tile([128, nt, d], F32, tag="k_nat")
            v_stage = nat.tile([128, nt, dv], F32, tag="v_stage")
            v_nat = nat.tile([128, nt, dv], F32R, tag="v_nat")
            nc.sync.dma_start(out=q_nat[:], in_=q_ap[n])
            nc.sync.dma_start(out=k_nat[:], in_=k_ap[n])
            # ones column for the softmax-denominator trick: pre-fill the
            # staging tile with 1.0, then DMA V over the data region
            nc.vector.memset(v_stage[:], 1.0)
            nc.sync.dma_start(out=v_stage[:, :, 0:d], in_=v_ap[n])
            nc.vector.tensor_copy(out=v_nat[:], in_=v_stage[:])

            qt = dmaj.tile([d, seq], F32R, tag="qt")
            kt = dmaj.tile([d, seq], F32R, tag="kt")
            for t in range(nt):
                tpq = tp_ps.tile([d, 128], F32, tag="tp")
                nc.tensor.transpose(tpq[:], q_nat[:, t, :], ident[:])
                nc.vector.tensor_copy(out=qt[:, t * 128 : (t + 1) * 128], in_=tpq[:])
                tpk = tp_ps.tile([d, 128], F32, tag="tp")
                nc.tensor.transpose(tpk[:], k_nat[:, t, :], ident[:])
                nc.vector.tensor_copy(out=kt[:, t * 128 : (t + 1) * 128], in_=tpk[:])

            oacc = oa_ps.tile([dv, seq], F32, tag="oacc")
            for jb in range(nt):
                st = st_ps.tile([128, seq], F32, tag="st")
                for c in range(n512):
                    nc.tensor.matmul(
                        st[:, c * nw : (c + 1) * nw],
                        kt[:, jb * 128 : (jb + 1) * 128],
                        qt[:, c * nw : (c + 1) * nw],
                        start=True,
                        stop=True,
                    )
                pt = ptp.tile([128, seq], F32R, tag="pt")
                nc.scalar.activation(
                    out=pt[:],
                    in_=st[:],
                    func=mybir.ActivationFunctionType.Exp,
                    scale=scale,
                )
                for c in range(n512):
                    nc.tensor.matmul(
                        oacc[:, c * nw : (c + 1) * nw],
                        v_nat[:, jb, :],
                        pt[:, c * nw : (c + 1) * nw],
                        start=(jb == 0),
                        stop=(jb == nt - 1),
                    )

            ot = otp.tile([dv, seq], F32, tag="ot")
            nc.vector.tensor_copy(out=ot[:], in_=oacc[:])
            ostage = outp.tile([128, nt, d], F32, tag="ostage")
            for t in range(nt):
                ob = tp_ps.tile([128, dv], F32, tag="tp")
                nc.tensor.transpose(
                    ob[:], ot[:, t * 128 : (t + 1) * 128], ident[0:dv, 0:dv]
                )
                rec = small.tile([128, 1], F32, tag="rec")
                nc.vector.reciprocal(out=rec[:], in_=ob[:, d : d + 1])
                nc.vector.tensor_scalar_mul(ostage[:, t, :], ob[:, 0:d], rec[:])
            nc.sync.dma_start(out=o_ap[:, :, n * d : (n + 1) * d], in_=ostage[:])

    _split_multi_waits(nc, mybir)
    return nc


def _get_nc():
    if "nc" not in _cache:
        _cache["nc"] = _build_nc()
    return _cache["nc"]


def _run(q, k, v, trace=False):
    from concourse.bass_utils import run_bass_kernel_spmd

    b, heads, h, w, d = 8, 8, 32, 32, 64
    q = np.ascontiguousarray(np.asarray(q, dtype=np.float32))
    k = np.ascontiguousarray(np.asarray(k, dtype=np.float32))
    v = np.ascontiguousarray(np.asarray(v, dtype=np.float32))
    assert q.shape == (b, heads, h, w, d), q.shape

    nc = _get_nc()
    in_maps = [
        {
            "q": q[c].reshape(heads, h * w, d),
            "k": k[c].reshape(heads, h * w, d),
            "v": v[c].reshape(heads, h * w, d),
        }
        for c in range(b)
    ]
    res = run_bass_kernel_spmd(nc, in_maps, core_ids=list(range(b)), trace=trace)
    out = np.stack(
        [res.results[c]["out"].reshape(h, w, heads * d) for c in range(b)]
    )
    return out, res


def kernel(q, k, v):
    out, _ = _run(q, k, v)
    return out


# revision 5
# speedup vs baseline: 1.5050x; 1.5050x over previous
"""Trainium2 Bass kernel for multi-head quadratic spatial attention.

Problem: q,k,v [b=8, heads=8, h=32, w=32, d=64] fp32; full attention over
the 1024-position spatial grid independently per (b, head); output
[b, h, w, heads*d].

Sharding: data-parallel over batch — core c handles b=c (8 heads of
[1024, 64] attention per core), no cross-core communication.

Per-core pipeline (matmuls in bf16 with fp32 PSUM accumulation; the
normalization path stays fp32):
  - load Q,K,V natural [1024, 64] as [128, 8, 64] SBUF tiles, cast to
    bf16 during the SWDGE DMA
  - PE-transpose Q,K tiles to d-major Qt,Kt [64, 1024] (identity
    matmul), 4 transposes grouped per [64, 512] PSUM tile so the DVE
    copies move full-width blocks
  - mm1: lhsT = Kt j-block [64, 128], rhs = Qt [64, 512] -> St PSUM
    [128 j, 1024 i] fp32  (St = S^T so softmax reduction lands on PE)
  - ScalarE exp(St * d^-1/2) -> Pt SBUF bf16 (scale folded in)
  - mm2: lhsT = [V | 1] j-chunk [128, 65] bf16, rhs = Pt -> accumulate
    PSUM Ot [65, 1024] fp32; row 64 = softmax denominators
  - PE-transpose Ot back per i-block (fp32), DVE reciprocal +
    tensor_scalar mult, DMA out natural layout
"""

from contextlib import ExitStack

import numpy as np

F32 = None
BF16 = None

_cache = {}


def _imports():
    global F32, BF16
    import concourse.bass as bass
    import concourse.tile as tile
    from concourse import mybir
    from concourse.masks import make_identity

    F32 = mybir.dt.float32
    BF16 = mybir.dt.bfloat16
    return bass, tile, mybir, make_identity


def _split_multi_waits(nc, mybir):
    """Walrus in this container supports only ONE sync-wait per instruction.
    Hoist extra waits onto same-engine InstNoOp's inserted just before."""
    ctr = 0
    for f in nc.m.functions:
        for bb in f.blocks:
            insts = bb.instructions
            if not any(
                i.sync_info and i.sync_info.on_wait and len(i.sync_info.on_wait) > 1
                for i in insts
            ):
                continue
            out = []
            for inst in insts:
                si = inst.sync_info
                waits = list(si.on_wait) if si and si.on_wait else []
                if len(waits) > 1:
                    for w in waits[:-1]:
                        ctr += 1
                        nop = mybir.InstNoOp(
                            name=f"I-wsplit-{ctr}",
                            engine=inst.engine,
                            ins=[],
                            outs=[],
                            sync_info=mybir.SyncInfo(on_wait=[w], on_update=[]),
                        )
                        nc.register_instruction(nop)
                        out.append(nop)
                    si.on_wait = waits[-1:]
                out.append(inst)
            bb.instructions = out


def _build_nc(heads=8, seq=1024, d=64):
    bass, tile, mybir, make_identity = _imports()
    nt = seq // 128
    dv = d + 1
    scale = float(d) ** -0.5
    n512 = max(1, seq // 512)
    nw = min(512, seq)
    tg = min(4, nt)  # transposes grouped per PSUM tile

    nc = bass.Bass(trn_type="TRN2", target_bir_lowering=False)
    q_d = nc.dram_tensor("q", [heads, seq, d], F32, kind="ExternalInput")
    k_d = nc.dram_tensor("k", [heads, seq, d], F32, kind="ExternalInput")
    v_d = nc.dram_tensor("v", [heads, seq, d], F32, kind="ExternalInput")
    o_d = nc.dram_tensor("out", [seq, heads * d], F32, kind="ExternalOutput")

    q_ap = q_d[:].rearrange("n (t p) d -> n p t d", p=128)
    k_ap = k_d[:].rearrange("n (t p) d -> n p t d", p=128)
    v_ap = v_d[:].rearrange("n (t p) d -> n p t d", p=128)
    o_ap = o_d[:].rearrange("(t p) c -> p t c", p=128)

    with tile.TileContext(nc) as tc, ExitStack() as ctx:
        consts = ctx.enter_context(tc.tile_pool(name="consts", bufs=1))
        nat = ctx.enter_context(tc.tile_pool(name="nat", bufs=2))
        dmaj = ctx.enter_context(tc.tile_pool(name="dmaj", bufs=2))
        ptp = ctx.enter_context(tc.tile_pool(name="ptp", bufs=3))
        otp = ctx.enter_context(tc.tile_pool(name="otp", bufs=2))
        outp = ctx.enter_context(tc.tile_pool(name="outp", bufs=2))
        small = ctx.enter_context(tc.tile_pool(name="small", bufs=4))

        tp_ps = ctx.enter_context(tc.tile_pool(name="tp_ps", bufs=2, space="PSUM"))
        st_ps = ctx.enter_context(tc.tile_pool(name="st_ps", bufs=2, space="PSUM"))
        oa_ps = ctx.enter_context(tc.tile_pool(name="oa_ps", bufs=1, space="PSUM"))

        ident_bf = consts.tile([128, 128], BF16)
        make_identity(nc, ident_bf[:])
        ident_f32 = consts.tile([128, 128], F32)
        make_identity(nc, ident_f32[:])

        for n in range(heads):
            q_nat = nat.tile([128, nt, d], BF16, tag="q_nat")
            k_nat = nat.tile([128, nt, d], BF16, tag="k_nat")
            v_nat = nat.tile([128, nt, dv], BF16, tag="v_nat")
            nc.gpsimd.dma_start(out=q_nat[:], in_=q_ap[n])
            nc.gpsimd.dma_start(out=k_nat[:], in_=k_ap[n])
            # ones column for the softmax-denominator trick: pre-fill with
            # 1.0, then DMA V (with bf16 cast) over the data region
            nc.vector.memset(v_nat[:], 1.0)
            nc.gpsimd.dma_start(out=v_nat[:, :, 0:d], in_=v_ap[n])

            # d-major Q/K, transposes grouped tg per PSUM tile
            qt = dmaj.tile([d, seq], BF16, tag="qt")
            kt = dmaj.tile([d, seq], BF16, tag="kt")
            for src, dst in ((q_nat, qt), (k_nat, kt)):
                for g in range(nt // tg):
                    tp = tp_ps.tile([d, tg * 128], BF16, tag="tp")
                    for u in range(tg):
                        nc.tensor.transpose(
                            tp[:, u * 128 : (u + 1) * 128],
                            src[:, g * tg + u, :],
                            ident_bf[:],
                        )
                    nc.vector.tensor_copy(
                        out=dst[:, g * tg * 128 : (g + 1) * tg * 128], in_=tp[:]
                    )

            oacc = oa_ps.tile([dv, seq], F32, tag="oacc")
            for jb in range(nt):
                st = st_ps.tile([128, seq], F32, tag="st")
                for c in range(n512):
                    nc.tensor.matmul(
                        st[:, c * nw : (c + 1) * nw],
                        kt[:, jb * 128 : (jb + 1) * 128],
                        qt[:, c * nw : (c + 1) * nw],
                        start=True,
                        stop=True,
                    )
                pt = ptp.tile([128, seq], BF16, tag="pt")
                nc.scalar.activation(
                    out=pt[:],
                    in_=st[:],
                    func=mybir.ActivationFunctionType.Exp,
                    scale=scale,
                )
                for c in range(n512):
                    nc.tensor.matmul(
                        oacc[:, c * nw : (c + 1) * nw],
                        v_nat[:, jb, :],
                        pt[:, c * nw : (c + 1) * nw],
                        start=(jb == 0),
                        stop=(jb == nt - 1),
                    )

            # epilogue (fp32): transpose Ot back, normalize, store
            ot = otp.tile([dv, seq], F32, tag="ot")
            nc.vector.tensor_copy(out=ot[:], in_=oacc[:])
            ostage = outp.tile([128, nt, d], F32, tag="ostage")
            for t in range(nt):
                ob = tp_ps.tile([128, dv], F32, tag="tp")
                nc.tensor.transpose(
                    ob[:], ot[:, t * 128 : (t + 1) * 128], ident_f32[0:dv, 0:dv]
                )
                rec = small.tile([128, 1], F32, tag="rec")
                nc.vector.reciprocal(out=rec[:], in_=ob[:, d : d + 1])
                nc.vector.tensor_scalar_mul(ostage[:, t, :], ob[:, 0:d], rec[:])
            nc.sync.dma_start(out=o_ap[:, :, n * d : (n + 1) * d], in_=ostage[:])

    _split_multi_waits(nc, mybir)
    return nc


def _get_nc():
    if "nc" not in _cache:
        _cache["nc"] = _build_nc()
    return _cache["nc"]


def _run(q, k, v, trace=False):
    from concourse.bass_utils import run_bass_kernel_spmd

    b, heads, h, w, d = 8, 8, 32, 32, 64
    q = np.ascontiguousarray(np.asarray(q, dtype=np.float32))
    k = np.ascontiguousarray(np.asarray(k, dtype=np.float32))
    v = np.ascontiguousarray(np.asarray(v, dtype=np.float32))
    assert q.shape == (b, heads, h, w, d), q.shape

    nc = _get_nc()
    in_maps = [
        {
            "q": q[c].reshape(heads, h * w, d),
            "k": k[c].reshape(heads, h * w, d),
            "v": v[c].reshape(heads, h * w, d),
        }
        for c in range(b)
    ]
    res = run_bass_kernel_spmd(nc, in_maps, core_ids=list(range(b)), trace=trace)
    out = np.stack(
        [res.results[c]["out"].reshape(h, w, heads * d) for c in range(b)]
    )
    return out, res


def kernel(q, k, v):
    out, _ = _run(q, k, v)
    return out
